# revision 36
# baseline (speedup 1.0000x reference)
"""Causal self-attention on 8 Trainium2 NeuronCores (Bass/Tile) — v3.

Problem: nn_CausalSelfAttention (B=4, T=2048, C=1024, H=16 heads, fp32).

Sharding: tensor-parallel over heads for QKV projection + attention
(2 heads per core), per-batch AllGather of attention outputs (fp16,
transposed layout, split collectives), then tensor-parallel over
output columns for the final projection.

v3 structure: software-pipelined across batches at MM granularity.
The attention of batch b is exp(ACT)-bound, so the QKV projection of
batch b+1, the V transposes of batch b+1 and the final projection of
batch b-1 are emitted as small "filler" quanta interleaved into
attention(b)'s kt loop — the PE fills its exp-wait slack with them and
the HAM clock gate stays warm.

Both heads' score tiles live in ONE 2-bank PSUM tile: the pair of
K=64 row-tiled matmuls (tile_position (0,0)/(64,0)) issues with a
single slot-wait and runs CONCURRENTLY in the PE array (measured
109.6 ns/MM vs 219 serial), and one batched exp covers both heads
(amortizes the ~293ns ACT instruction overhead).

Layouts (feature dim on partitions everywhere):
  xT      [C, B*T] fp16   input, replicated to all cores
  Q^T,K^T [CH, B*T] fp16  head hh occupies partitions hh*64..hh*64+64
  V       [B*T, CH] fp16  tiled with a ones-column per head (denominator)
  S^T     [kr, 2*q] tiles scores transposed, both heads side by side
  attn^T  [CH, T] fp16    per-core, per-batch -> split AllGathers
                          (2 halves; last batch 4 quarters so the tail
                          collective is tiny)
  y^T     [OC, B*T]       per-core 128-column slice of the final output

Softmax: unnormalized exp (scores are O(1), no max subtraction);
causal mask = PE add of a -60000 upper-triangular [128,128] constant on
diagonal-crossing tiles; denominator from the V ones-column; PV tile is
evacuated to SBUF at q-tile end (frees its PSUM bank pair), then DVE
reciprocal_approx_fast + PE broadcast + DVE multiply normalize it; all
evacuations/bias adds run on DVE so ACT does nothing but exp.
"""

import numpy as np
from contextlib import ExitStack

P = 128
NQ = 512  # q/moving-operand tile width


def build_attention_nc(B, T, C, H, n_cores):
    import concourse.bass as bass  # noqa: F401
    import concourse.bacc as bacc
    import concourse.tile as tile
    import concourse.mybir as mybir

    f32 = mybir.dt.float32
    fp16 = mybir.dt.float16
    Exp = mybir.ActivationFunctionType.Exp

    hs = C // H              # head size
    hpc = H // n_cores       # heads per core
    CH = hpc * hs            # qkv channels per core
    OC = C // n_cores        # output columns per core
    NT = B * T               # tokens
    KT_E = C // P            # contraction tiles over embedding dim
    TQ = T // NQ             # q tiles per batch
    TK = T // P              # kr tiles per batch
    TKALL = NT // P          # kr tiles over all batches
    DPB = NQ // P            # kr-tiles crossing one q-tile's diagonal
    WV = hpc * (hs + 1)      # V storage width per kr-tile (with ones cols)
    NQ2 = 2 * NQ

    assert T % NQ == 0 and C % P == 0 and NT % NQ == 0
    assert CH == P and H % n_cores == 0 and hpc == 2 and hs == 64
    scale = 1.0 / float(np.sqrt(hs))

    nc = bacc.Bacc("TRN2", target_bir_lowering=False, debug=False,
                   num_devices=n_cores)

    xT = nc.dram_tensor("xT", [C, NT], fp16, kind="ExternalInput")
    wqkv = nc.dram_tensor("wqkv", [C, 3 * CH], fp16, kind="ExternalInput")
    bqkv = nc.dram_tensor("bqkv", [CH, 3], f32, kind="ExternalInput")
    wp = nc.dram_tensor("wp", [C, OC], fp16, kind="ExternalInput")
    bp = nc.dram_tensor("bp", [OC, 1], f32, kind="ExternalInput")
    yT = nc.dram_tensor("yT", [OC, NT], f32, kind="ExternalOutput")

    # constants baked into the NEFF
    ident_np = np.eye(P, dtype=np.float16)
    # 0/1 keep-mask: position (kr, q) allowed iff kr <= q within the
    # diagonal block (applied multiplicatively on the exp'd scores by DVE)
    mask_np = np.where(
        np.arange(P)[:, None] > np.arange(P)[None, :],
        np.float16(0.0), np.float16(1.0)).astype(np.float16)
    ident_dram = nc.inline_tensor(ident_np, name="ident_const")
    mask_dram = nc.inline_tensor(mask_np, name="mask_const")
    ones_dram = nc.inline_tensor(np.ones((P, hs), dtype=np.float16),
                                 name="ones_const")
    vones_dram = nc.inline_tensor(np.ones((P, TKALL * hpc), dtype=np.float16),
                                  name="vones_const")

    with tile.TileContext(nc) as tc, ExitStack() as ctx:
        const = ctx.enter_context(tc.tile_pool(name="const", bufs=1))
        big = ctx.enter_context(tc.tile_pool(name="big", bufs=1))
        xin = ctx.enter_context(tc.tile_pool(name="xin", bufs=4))
        evac = ctx.enter_context(tc.tile_pool(name="evac", bufs=3))
        pexp = ctx.enter_context(tc.tile_pool(name="pexp", bufs=6))
        # PSUM: stp 2x2 banks (S pairs) + qkp 2x1 (QKV/proj/transpose/bc)
        #       + pvp 1x2 (PV pair) = 8 banks
        stp = ctx.enter_context(tc.tile_pool(name="stp", bufs=2, space="PSUM"))
        qkp = ctx.enter_context(tc.tile_pool(name="qkp", bufs=2, space="PSUM"))
        pvp = ctx.enter_context(tc.tile_pool(name="pvp", bufs=1, space="PSUM"))
        dram = ctx.enter_context(tc.tile_pool(name="dram", bufs=1,
                                              space="DRAM"))

        ident_t = const.tile([P, P], fp16)
        mask_sb = const.tile([P, P], fp16)
        ones_sb = const.tile([P, hs], fp16)
        bqkv_sb = const.tile([CH, 3], f32)
        bp_sb = const.tile([OC, 1], f32)
        w_sb = const.tile([P, KT_E * 3 * CH], fp16)
        wp_sb = const.tile([P, KT_E * OC], fp16)

        # critical-path DMAs first: the first QKV matmul only needs w_sb
        # (and its own x tile); everything else can trail
        nc.sync.dma_start(
            w_sb[:].rearrange("p (k m) -> p k m", k=KT_E),
            wqkv[:].rearrange("(k p) m -> p k m", p=P),
        )
        nc.sync.dma_start(bqkv_sb[:], bqkv[:])
        nc.sync.dma_start(ident_t[:], ident_dram[:])
        nc.sync.dma_start(mask_sb[:], mask_dram[:])
        nc.sync.dma_start(ones_sb[:], ones_dram[:])
        nc.sync.dma_start(bp_sb[:], bp[:])
        nc.sync.dma_start(
            wp_sb[:].rearrange("p (k m) -> p k m", k=KT_E),
            wp[:].rearrange("(k p) m -> p k m", p=P),
        )

        QT = big.tile([CH, NT], fp16)
        KT = big.tile([CH, NT], fp16)
        VT = big.tile([CH, NT], fp16)
        Vn = big.tile([P, TKALL * WV], fp16)

        # ones columns of V (softmax denominator trick) — memset on-chip
        # (a DMA here would be a 16K-descriptor 2-byte scatter, ~10us)
        ones_view = Vn[:].rearrange("p (v h d) -> p v h d", h=hpc, d=hs + 1)[
            :, :, :, hs:hs + 1
        ]
        nc.any.memset(ones_view, 1.0)

        # ---- QKV projection + V transpose quanta --------------------------
        # one "quantum" is a small closure emitting ~0.4-1.7us of PE work;
        # quanta are either emitted densely (prologue) or injected into an
        # attention kt loop as fillers.

        def qkv_row_quanta(n, pools):
            """Quanta computing Q/K/V^T for global row-tile n (NQ tokens)."""
            ns = n * NQ
            state = {}

            def dma_x():
                xt = xin.tile([P, KT_E * NQ], fp16, tag="xcol")
                nc.sync.dma_start(
                    xt[:].rearrange("p (k q) -> p k q", k=KT_E),
                    xT[:, ns:ns + NQ].rearrange("(k p) q -> p k q", p=P),
                )
                state["xt"] = xt

            quanta = [dma_x]
            for m in range(3):
                def mm_a(m=m):
                    pl, tg = pools[m % len(pools)]
                    ps = pl.tile([P, NQ], f32, tag=tg)
                    state[m] = ps
                    xt = state["xt"]
                    for k in range(4):
                        nc.tensor.matmul(
                            ps[:],
                            lhsT=w_sb[:, k * 3 * CH + m * CH:
                                      k * 3 * CH + (m + 1) * CH],
                            rhs=xt[:, k * NQ:(k + 1) * NQ],
                            start=(k == 0), stop=False,
                        )

                def mm_b(m=m):
                    ps = state[m]
                    xt = state["xt"]
                    for k in range(4, KT_E):
                        nc.tensor.matmul(
                            ps[:],
                            lhsT=w_sb[:, k * 3 * CH + m * CH:
                                      k * 3 * CH + (m + 1) * CH],
                            rhs=xt[:, k * NQ:(k + 1) * NQ],
                            start=False, stop=(k == KT_E - 1),
                        )
                    dst = (QT, KT, VT)[m]
                    nc.vector.tensor_scalar_add(dst[:, ns:ns + NQ], ps[:],
                                                bqkv_sb[:, m:m + 1])
                quanta += [mm_a, mm_b]
            return quanta

        def vtrans_quanta(n, pools):
            """Transpose row-tile n's V^T into Vn (with ones cols kept)."""
            state = {}

            def tr_a():
                pl, tg = pools[0]
                tp = pl.tile([P, DPB * CH], fp16, tag=tg, name="tp")
                state["tp"] = tp
                for j in range(2):
                    nc.tensor.transpose(
                        tp[:, j * CH:(j + 1) * CH],
                        VT[:, (n * DPB + j) * P:(n * DPB + j + 1) * P],
                        ident_t[:],
                    )

            def tr_b():
                tp = state["tp"]
                for j in range(2, DPB):
                    nc.tensor.transpose(
                        tp[:, j * CH:(j + 1) * CH],
                        VT[:, (n * DPB + j) * P:(n * DPB + j + 1) * P],
                        ident_t[:],
                    )
                vi0 = n * DPB
                dst = Vn[:, vi0 * WV:(vi0 + DPB) * WV].rearrange(
                    "p (v h d) -> p v h d", h=hpc, d=hs + 1
                )[:, :, :, 0:hs]
                nc.vector.tensor_copy(dst, tp[:].rearrange(
                    "p (v h d) -> p v h d", h=hpc, d=hs))

            return [tr_a, tr_b]

        def qkv_batch_quanta(b, pools):
            quanta = []
            for r in range(TQ):
                quanta += qkv_row_quanta(b * TQ + r, pools)
                quanta += vtrans_quanta(b * TQ + r, pools)
            return quanta

        # ---- AllGather + proj ---------------------------------------------
        # per-batch contiguous column parts: early batches use one full-T
        # AllGather (fewer ~16us collective floors on the serial CC queue),
        # later batches split progressively so the tail collective is tiny
        T2 = T // 2
        if B == 4:
            parts_w = [[T], [T], [T2, T2], [T2, NQ, NQ]]
        else:
            parts_w = [[T2, T2] for _ in range(B)]
        parts_lo = [list(np.cumsum([0] + w[:-1])) for w in parts_w]

        def part_of(b, col):
            for p in range(len(parts_w[b]) - 1, -1, -1):
                if col >= parts_lo[b][p]:
                    return p, col - parts_lo[b][p]
            raise AssertionError

        cc_ins = [
            [dram.tile([CH, w], fp16, name=f"ccin{b}p{p}")
             for p, w in enumerate(parts_w[b])]
            for b in range(B)
        ]
        cc_outs = [
            [dram.tile([n_cores * CH, w], fp16, addr_space="Shared",
                       name=f"ccout{b}p{p}") for p, w in enumerate(parts_w[b])]
            for b in range(B)
        ]

        def issue_ag(b, p):
            nc.gpsimd.collective_compute(
                "AllGather",
                mybir.AluOpType.bypass,
                replica_groups=[list(range(n_cores))],
                ins=[cc_ins[b][p][:].opt()],
                outs=[cc_outs[b][p][:].opt()],
            )

        def proj_quanta(b, pools):
            """Final projection of batch b (reads gathered attn parts)."""
            quanta = []
            for n in range(TQ):
                ns = n * NQ
                state = {}

                def dma_r(b=b, ns=ns, state=state):
                    p, off = part_of(b, ns)
                    src = cc_outs[b][p]
                    rt = xin.tile([P, KT_E * NQ], fp16, tag="xcol")
                    nc.sync.dma_start(
                        rt[:].rearrange("p (k q) -> p k q", k=KT_E),
                        src[:, off:off + NQ].rearrange("(k p) q -> p k q",
                                                       p=P),
                    )
                    state["rt"] = rt

                def mm_a(state=state):
                    pl, tg = pools[0]
                    ps = pl.tile([P, NQ], f32, tag=tg)
                    state["ps"] = ps
                    rt = state["rt"]
                    for k in range(4):
                        nc.tensor.matmul(
                            ps[0:OC, :],
                            lhsT=wp_sb[:, k * OC:(k + 1) * OC],
                            rhs=rt[:, k * NQ:(k + 1) * NQ],
                            start=(k == 0), stop=False,
                        )

                def mm_b(b=b, ns=ns, state=state):
                    ps = state["ps"]
                    rt = state["rt"]
                    for k in range(4, KT_E):
                        nc.tensor.matmul(
                            ps[0:OC, :],
                            lhsT=wp_sb[:, k * OC:(k + 1) * OC],
                            rhs=rt[:, k * NQ:(k + 1) * NQ],
                            start=False, stop=(k == KT_E - 1),
                        )
                    yo = evac.tile([OC, NQ], f32, tag="yo")
                    nc.vector.tensor_scalar_add(yo[:], ps[0:OC, :],
                                                bp_sb[:, 0:1])
                    nc.sync.dma_start(yT[:, b * T + ns:b * T + ns + NQ],
                                      yo[:])

                quanta += [dma_r, mm_a, mm_b]
            return quanta

        # ---- attention with filler injection ------------------------------
        def attention(b, fillers, on_norm_done=None):
            prev = None

            def emit_fillers(k, reserve=4):
                # keep a few quanta in reserve so the pre-final-norm drain
                # has PE work to cover the DVE recip latency
                for _ in range(k):
                    if len(fillers) <= reserve:
                        return
                    fillers.pop(0)()

            def issue_evac_recip(state):
                # evacuate the PV pair to SBUF (frees its PSUM banks) and
                # start the reciprocal of the two denominator rows
                qt0, pv2, sb = state
                pvsb, rf, r16 = sb
                nc.vector.tensor_copy(pvsb[:], pv2[:])
                # approx-recip mishandles single-row APs at base partition
                # 64; run it over the whole 65-row slab (lanes independent,
                # junk in rows 0..63 never read)
                nc.vector.reciprocal_approx_fast(rf[0:hs + 1, :],
                                                 pvsb[0:hs + 1, :])
                nc.vector.tensor_copy(r16[hs:hs + 1, :], rf[hs:hs + 1, :])

            def issue_norm_rest(state):
                qt0, pv2, sb = state
                pvsb, rf, r16 = sb
                for hh in range(hpc):
                    bc = qkp.tile([P, NQ], f32, tag="qk", name="bc")
                    nc.tensor.matmul(
                        bc[0:hs, :],
                        lhsT=ones_sb[hs:hs + 1, :],
                        rhs=r16[hs:hs + 1, hh * NQ:(hh + 1) * NQ],
                        start=True,
                        stop=True,
                    )
                    ao = evac.tile([hs, NQ], fp16, tag="ao")
                    nc.vector.tensor_mul(
                        ao[:], bc[0:hs, :],
                        pvsb[0:hs, hh * NQ:(hh + 1) * NQ])
                    p, c = part_of(b, qt0 * NQ)
                    nc.sync.dma_start(
                        cc_ins[b][p][hh * hs:(hh + 1) * hs, c:c + NQ],
                        ao[:],
                    )
                if on_norm_done is not None:
                    p, c = part_of(b, qt0 * NQ)
                    if c + NQ == parts_w[b][p]:
                        on_norm_done(b, p)

            for qt in range(TQ):
                qs = b * T + qt * NQ
                nkt = DPB * qt + DPB
                pv2 = pvp.tile([hs + 1, NQ2], f32, tag="pv", name="pv2")
                pes = {}

                def issue_st(kt, qt=qt, qs=qs, pes=pes):
                    ks = b * T + kt * P
                    diag = kt >= DPB * qt
                    j = kt - DPB * qt
                    c0 = j * P if diag else 0
                    st2 = stp.tile([P, NQ2], f32, tag="st", name="st2")
                    for hh in range(hpc):
                        nc.tensor.matmul(
                            st2[:, hh * NQ + c0:(hh + 1) * NQ],
                            lhsT=KT[hh * hs:(hh + 1) * hs, ks:ks + P],
                            rhs=QT[hh * hs:(hh + 1) * hs,
                                   qs + c0:qs + NQ],
                            start=True,
                            stop=True,
                            tile_position=(hh * hs, 0),
                        )
                    pe2 = pexp.tile([P, NQ2], fp16, tag="pe", name="pe2")
                    if c0 == 0:
                        nc.scalar.activation(pe2[:], st2[:], Exp, scale=scale)
                    else:
                        nc.scalar.activation(
                            pe2[:].rearrange("p (h q) -> p h q",
                                             h=hpc)[:, :, c0:NQ],
                            st2[:].rearrange("p (h q) -> p h q",
                                             h=hpc)[:, :, c0:NQ],
                            Exp, scale=scale)
                    if diag:
                        # causal mask: zero the strictly-lower triangle of
                        # the diagonal [128,128] block on DVE (keeps the PE
                        # free of mask matmuls); masked exps contribute 0
                        # to both PV and the ones-column denominator
                        for hh in range(hpc):
                            blk = pe2[:, hh * NQ + c0:hh * NQ + c0 + P]
                            nc.vector.tensor_mul(blk, blk, mask_sb[:])
                    pes[kt] = (pe2, c0)

                def issue_pv(kt, nkt=nkt, pv2=pv2, pes=pes):
                    vi = b * TK + kt
                    pe2, c0 = pes.pop(kt)
                    for hh in range(hpc):
                        nc.tensor.matmul(
                            pv2[:, hh * NQ + c0:(hh + 1) * NQ],
                            lhsT=Vn[:, vi * WV + hh * (hs + 1):
                                    vi * WV + (hh + 1) * (hs + 1)],
                            rhs=pe2[:, hh * NQ + c0:(hh + 1) * NQ],
                            start=(kt == 0),
                            stop=(kt == nkt - 1),
                        )

                for kt in range(nkt):
                    issue_st(kt)
                    if kt == 3 and prev is not None:
                        issue_norm_rest(prev)
                    if kt >= 2:
                        issue_pv(kt - 2)
                    emit_fillers(1)
                for kt in range(max(0, nkt - 2), nkt):
                    issue_pv(kt)

                sb = (evac.tile([hs + 1, NQ2], f32, tag="pvs", name="pvsb",
                                bufs=2),
                      evac.tile([P, NQ2], f32, tag="rec", name="rf", bufs=2),
                      evac.tile([P, NQ2], fp16, tag="rec16", name="r16",
                                bufs=2))
                prev = (qt, pv2, sb)
                issue_evac_recip(prev)

            # give the DVE recip chain time to finish before the final
            # broadcast matmul enters the PE stream
            emit_fillers(4, reserve=0)
            issue_norm_rest(prev)

        # ---- main schedule ------------------------------------------------
        # prologue: batch 0's QKV dense, alternating PSUM slots
        for q in qkv_batch_quanta(0, [(stp, "st"), (qkp, "qk")]):
            q()

        for b in range(B):
            fillers = []
            if b + 1 < B:
                fillers += qkv_batch_quanta(b + 1, [(qkp, "qk")])
            if b >= 1:
                fillers += proj_quanta(b - 1, [(qkp, "qk")])
            attention(b, fillers, on_norm_done=issue_ag)
            # drain leftover fillers densely
            while fillers:
                fillers.pop(0)()
        for q in proj_quanta(B - 1, [(qkp, "qk")]):
            q()

    nc.compile()
    return nc


def shard_inputs(x, W_qkv, b_qkv, W_proj, b_proj, H, n_cores):
    B, T, C = x.shape
    hs = C // H
    hpc = H // n_cores
    CH = hpc * hs
    OC = C // n_cores
    x2 = np.asarray(x, dtype=np.float32).reshape(B * T, C)
    xT = np.ascontiguousarray(x2.T.astype(np.float16))
    W_qkv = np.asarray(W_qkv, dtype=np.float32)
    b_qkv = np.asarray(b_qkv, dtype=np.float32)
    W_proj = np.asarray(W_proj, dtype=np.float32)
    b_proj = np.asarray(b_proj, dtype=np.float32)
    in_maps = []
    for i in range(n_cores):
        sl = slice(i * CH, (i + 1) * CH)
        wqkv_i = np.ascontiguousarray(np.concatenate(
            [W_qkv[:, sl], W_qkv[:, C:][:, sl], W_qkv[:, 2 * C:][:, sl]],
            axis=1).astype(np.float16))
        bqkv_i = np.ascontiguousarray(np.stack(
            [b_qkv[sl], b_qkv[C:][sl], b_qkv[2 * C:][sl]], axis=1))
        wp_i = np.ascontiguousarray(
            W_proj[:, i * OC:(i + 1) * OC].astype(np.float16))
        bp_i = np.ascontiguousarray(b_proj[i * OC:(i + 1) * OC].reshape(OC, 1))
        in_maps.append({"xT": xT, "wqkv": wqkv_i, "bqkv": bqkv_i,
                        "wp": wp_i, "bp": bp_i})
    return in_maps


def gather_output(results, B, T, C, n_cores):
    yT = np.concatenate([results[i]["yT"] for i in range(n_cores)], axis=0)
    return np.ascontiguousarray(yT.T).reshape(B, T, C).astype(np.float32)


_NC_CACHE = {}


def _get_nc(B, T, C, H, n_cores):
    key = (B, T, C, H, n_cores)
    if key not in _NC_CACHE:
        _NC_CACHE[key] = build_attention_nc(B, T, C, H, n_cores)
    return _NC_CACHE[key]


def kernel(x, W_qkv, b_qkv, W_proj, b_proj):
    from concourse import bass_utils

    B, T, C = 4, 2048, 1024
    H, n_cores = 16, 8
    assert x.shape == (B, T, C)
    nc = _get_nc(B, T, C, H, n_cores)
    in_maps = shard_inputs(x, W_qkv, b_qkv, W_proj, b_proj, H, n_cores)
    res = bass_utils.run_bass_kernel_spmd(
        nc, in_maps, core_ids=list(range(n_cores))
    )
    return gather_output(res.results, B, T, C, n_cores)


# revision 37
# speedup vs baseline: 1.0876x; 1.0876x over previous
"""Causal self-attention on 8 Trainium2 NeuronCores (Bass/Tile) — v3.

Problem: nn_CausalSelfAttention (B=4, T=2048, C=1024, H=16 heads, fp32).

Sharding: tensor-parallel over heads for QKV projection + attention
(2 heads per core), per-batch AllGather of attention outputs (fp16,
transposed layout, split collectives), then tensor-parallel over
output columns for the final projection.

v3 structure: software-pipelined across batches at MM granularity.
The attention of batch b is exp(ACT)-bound, so the QKV projection of
batch b+1, the V transposes of batch b+1 and the final projection of
batch b-1 are emitted as small "filler" quanta interleaved into
attention(b)'s kt loop — the PE fills its exp-wait slack with them and
the HAM clock gate stays warm.

Both heads' score tiles live in ONE 2-bank PSUM tile: the pair of
K=64 row-tiled matmuls (tile_position (0,0)/(64,0)) issues with a
single slot-wait and runs CONCURRENTLY in the PE array (measured
109.6 ns/MM vs 219 serial), and one batched exp covers both heads
(amortizes the ~293ns ACT instruction overhead).

Layouts (feature dim on partitions everywhere):
  xT      [C, B*T] fp16   input, replicated to all cores
  Q^T,K^T [CH, B*T] fp16  head hh occupies partitions hh*64..hh*64+64
  V       [B*T, CH] fp16  tiled with a ones-column per head (denominator)
  S^T     [kr, 2*q] tiles scores transposed, both heads side by side
  attn^T  [CH, T] fp16    per-core, per-batch -> split AllGathers
                          (2 halves; last batch 4 quarters so the tail
                          collective is tiny)
  y^T     [OC, B*T]       per-core 128-column slice of the final output

Softmax: unnormalized exp (scores are O(1), no max subtraction);
causal mask = PE add of a -60000 upper-triangular [128,128] constant on
diagonal-crossing tiles; denominator from the V ones-column; PV tile is
evacuated to SBUF at q-tile end (frees its PSUM bank pair), then DVE
reciprocal_approx_fast + PE broadcast + DVE multiply normalize it; all
evacuations/bias adds run on DVE so ACT does nothing but exp.
"""

import numpy as np
from contextlib import ExitStack

P = 128
NQ = 512  # q/moving-operand tile width


def build_attention_nc(B, T, C, H, n_cores):
    import concourse.bass as bass  # noqa: F401
    import concourse.bacc as bacc
    import concourse.tile as tile
    import concourse.mybir as mybir

    f32 = mybir.dt.float32
    fp16 = mybir.dt.float16
    Exp = mybir.ActivationFunctionType.Exp

    hs = C // H              # head size
    hpc = H // n_cores       # heads per core
    CH = hpc * hs            # qkv channels per core
    OC = C // n_cores        # output columns per core
    NT = B * T               # tokens
    KT_E = C // P            # contraction tiles over embedding dim
    TQ = T // NQ             # q tiles per batch
    TK = T // P              # kr tiles per batch
    TKALL = NT // P          # kr tiles over all batches
    DPB = NQ // P            # kr-tiles crossing one q-tile's diagonal
    WV = hpc * (hs + 1)      # V storage width per kr-tile (with ones cols)
    NQ2 = 2 * NQ

    assert T % NQ == 0 and C % P == 0 and NT % NQ == 0
    assert CH == P and H % n_cores == 0 and hpc == 2 and hs == 64
    scale = 1.0 / float(np.sqrt(hs))

    nc = bacc.Bacc("TRN2", target_bir_lowering=False, debug=False,
                   num_devices=n_cores)

    xT = nc.dram_tensor("xT", [C, NT], fp16, kind="ExternalInput")
    wqkv = nc.dram_tensor("wqkv", [C, 3 * CH], fp16, kind="ExternalInput")
    bqkv = nc.dram_tensor("bqkv", [CH, 3], f32, kind="ExternalInput")
    wp = nc.dram_tensor("wp", [C, OC], fp16, kind="ExternalInput")
    bp = nc.dram_tensor("bp", [OC, 1], f32, kind="ExternalInput")
    yT = nc.dram_tensor("yT", [OC, NT], f32, kind="ExternalOutput")

    # constants baked into the NEFF
    ident_np = np.eye(P, dtype=np.float16)
    # 0/1 keep-mask: position (kr, q) allowed iff kr <= q within the
    # diagonal block (applied multiplicatively on the exp'd scores by DVE)
    mask_np = np.where(
        np.arange(P)[:, None] > np.arange(P)[None, :],
        np.float16(0.0), np.float16(1.0)).astype(np.float16)
    ident_dram = nc.inline_tensor(ident_np, name="ident_const")
    mask_dram = nc.inline_tensor(mask_np, name="mask_const")
    ones_dram = nc.inline_tensor(np.ones((P, hs), dtype=np.float16),
                                 name="ones_const")
    vones_dram = nc.inline_tensor(np.ones((P, TKALL * hpc), dtype=np.float16),
                                  name="vones_const")

    with tile.TileContext(nc) as tc, ExitStack() as ctx:
        const = ctx.enter_context(tc.tile_pool(name="const", bufs=1))
        big = ctx.enter_context(tc.tile_pool(name="big", bufs=1))
        xin = ctx.enter_context(tc.tile_pool(name="xin", bufs=4))
        evac = ctx.enter_context(tc.tile_pool(name="evac", bufs=3))
        pexp = ctx.enter_context(tc.tile_pool(name="pexp", bufs=6))
        # PSUM: stp 2x2 banks (S pairs) + qkp 2x1 (QKV/proj/transpose/bc)
        #       + pvp 1x2 (PV pair) = 8 banks
        stp = ctx.enter_context(tc.tile_pool(name="stp", bufs=2, space="PSUM"))
        qkp = ctx.enter_context(tc.tile_pool(name="qkp", bufs=2, space="PSUM"))
        pvp = ctx.enter_context(tc.tile_pool(name="pvp", bufs=1, space="PSUM"))
        dram = ctx.enter_context(tc.tile_pool(name="dram", bufs=1,
                                              space="DRAM"))

        ident_t = const.tile([P, P], fp16)
        mask_sb = const.tile([P, P], fp16)
        ones_sb = const.tile([P, hs], fp16)
        bqkv_sb = const.tile([CH, 3], f32)
        bp_sb = const.tile([OC, 1], f32)
        w_sb = const.tile([P, KT_E * 3 * CH], fp16)
        wp_sb = const.tile([P, KT_E * OC], fp16)

        nc.sync.dma_start(ident_t[:], ident_dram[:])
        nc.sync.dma_start(mask_sb[:], mask_dram[:])
        nc.sync.dma_start(ones_sb[:], ones_dram[:])
        nc.sync.dma_start(bqkv_sb[:], bqkv[:])
        nc.sync.dma_start(bp_sb[:], bp[:])
        nc.sync.dma_start(
            w_sb[:].rearrange("p (k m) -> p k m", k=KT_E),
            wqkv[:].rearrange("(k p) m -> p k m", p=P),
        )
        nc.sync.dma_start(
            wp_sb[:].rearrange("p (k m) -> p k m", k=KT_E),
            wp[:].rearrange("(k p) m -> p k m", p=P),
        )

        QT = big.tile([CH, NT], fp16)
        KT = big.tile([CH, NT], fp16)
        VT = big.tile([CH, NT], fp16)
        Vn = big.tile([P, TKALL * WV], fp16)

        # ones columns of V (softmax denominator trick)
        ones_view = Vn[:].rearrange("p (v h d) -> p v h d", h=hpc, d=hs + 1)[
            :, :, :, hs:hs + 1
        ]
        nc.sync.dma_start(
            ones_view,
            vones_dram[:].rearrange("p (v h d) -> p v h d", h=hpc, d=1),
        )

        # ---- QKV projection + V transpose quanta --------------------------
        # one "quantum" is a small closure emitting ~0.4-1.7us of PE work;
        # quanta are either emitted densely (prologue) or injected into an
        # attention kt loop as fillers.

        def qkv_row_quanta(n, pools):
            """Quanta computing Q/K/V^T for global row-tile n (NQ tokens)."""
            ns = n * NQ
            state = {}

            def dma_x():
                xt = xin.tile([P, KT_E * NQ], fp16, tag="xcol")
                nc.sync.dma_start(
                    xt[:].rearrange("p (k q) -> p k q", k=KT_E),
                    xT[:, ns:ns + NQ].rearrange("(k p) q -> p k q", p=P),
                )
                state["xt"] = xt

            quanta = [dma_x]
            for m in range(3):
                def mm_a(m=m):
                    pl, tg = pools[m % len(pools)]
                    ps = pl.tile([P, NQ], f32, tag=tg)
                    state[m] = ps
                    xt = state["xt"]
                    for k in range(4):
                        nc.tensor.matmul(
                            ps[:],
                            lhsT=w_sb[:, k * 3 * CH + m * CH:
                                      k * 3 * CH + (m + 1) * CH],
                            rhs=xt[:, k * NQ:(k + 1) * NQ],
                            start=(k == 0), stop=False,
                        )

                def mm_b(m=m):
                    ps = state[m]
                    xt = state["xt"]
                    for k in range(4, KT_E):
                        nc.tensor.matmul(
                            ps[:],
                            lhsT=w_sb[:, k * 3 * CH + m * CH:
                                      k * 3 * CH + (m + 1) * CH],
                            rhs=xt[:, k * NQ:(k + 1) * NQ],
                            start=False, stop=(k == KT_E - 1),
                        )
                    dst = (QT, KT, VT)[m]
                    nc.vector.tensor_scalar_add(dst[:, ns:ns + NQ], ps[:],
                                                bqkv_sb[:, m:m + 1])
                quanta += [mm_a, mm_b]
            return quanta

        def vtrans_quanta(n, pools):
            """Transpose row-tile n's V^T into Vn (with ones cols kept)."""
            state = {}

            def tr_a():
                pl, tg = pools[0]
                tp = pl.tile([P, DPB * CH], fp16, tag=tg, name="tp")
                state["tp"] = tp
                for j in range(2):
                    nc.tensor.transpose(
                        tp[:, j * CH:(j + 1) * CH],
                        VT[:, (n * DPB + j) * P:(n * DPB + j + 1) * P],
                        ident_t[:],
                    )

            def tr_b():
                tp = state["tp"]
                for j in range(2, DPB):
                    nc.tensor.transpose(
                        tp[:, j * CH:(j + 1) * CH],
                        VT[:, (n * DPB + j) * P:(n * DPB + j + 1) * P],
                        ident_t[:],
                    )
                vi0 = n * DPB
                dst = Vn[:, vi0 * WV:(vi0 + DPB) * WV].rearrange(
                    "p (v h d) -> p v h d", h=hpc, d=hs + 1
                )[:, :, :, 0:hs]
                nc.vector.tensor_copy(dst, tp[:].rearrange(
                    "p (v h d) -> p v h d", h=hpc, d=hs))

            return [tr_a, tr_b]

        def qkv_batch_quanta(b, pools):
            quanta = []
            for r in range(TQ):
                quanta += qkv_row_quanta(b * TQ + r, pools)
                quanta += vtrans_quanta(b * TQ + r, pools)
            return quanta

        # ---- AllGather + proj ---------------------------------------------
        # per-batch contiguous column parts: early batches use one full-T
        # AllGather (fewer ~16us collective floors on the serial CC queue),
        # later batches split progressively so the tail collective is tiny
        T2 = T // 2
        if B == 4:
            parts_w = [[T], [T], [T2, T2], [T2, NQ, NQ]]
        else:
            parts_w = [[T2, T2] for _ in range(B)]
        parts_lo = [list(np.cumsum([0] + w[:-1])) for w in parts_w]

        def part_of(b, col):
            for p in range(len(parts_w[b]) - 1, -1, -1):
                if col >= parts_lo[b][p]:
                    return p, col - parts_lo[b][p]
            raise AssertionError

        cc_ins = [
            [dram.tile([CH, w], fp16, name=f"ccin{b}p{p}")
             for p, w in enumerate(parts_w[b])]
            for b in range(B)
        ]
        cc_outs = [
            [dram.tile([n_cores * CH, w], fp16, addr_space="Shared",
                       name=f"ccout{b}p{p}") for p, w in enumerate(parts_w[b])]
            for b in range(B)
        ]

        def issue_ag(b, p):
            nc.gpsimd.collective_compute(
                "AllGather",
                mybir.AluOpType.bypass,
                replica_groups=[list(range(n_cores))],
                ins=[cc_ins[b][p][:].opt()],
                outs=[cc_outs[b][p][:].opt()],
            )

        def proj_quanta(b, pools):
            """Final projection of batch b (reads gathered attn parts)."""
            quanta = []
            for n in range(TQ):
                ns = n * NQ
                state = {}

                def dma_r(b=b, ns=ns, state=state):
                    p, off = part_of(b, ns)
                    src = cc_outs[b][p]
                    rt = xin.tile([P, KT_E * NQ], fp16, tag="xcol")
                    nc.sync.dma_start(
                        rt[:].rearrange("p (k q) -> p k q", k=KT_E),
                        src[:, off:off + NQ].rearrange("(k p) q -> p k q",
                                                       p=P),
                    )
                    state["rt"] = rt

                def mm_a(state=state):
                    pl, tg = pools[0]
                    ps = pl.tile([P, NQ], f32, tag=tg)
                    state["ps"] = ps
                    rt = state["rt"]
                    for k in range(4):
                        nc.tensor.matmul(
                            ps[0:OC, :],
                            lhsT=wp_sb[:, k * OC:(k + 1) * OC],
                            rhs=rt[:, k * NQ:(k + 1) * NQ],
                            start=(k == 0), stop=False,
                        )

                def mm_b(b=b, ns=ns, state=state):
                    ps = state["ps"]
                    rt = state["rt"]
                    for k in range(4, KT_E):
                        nc.tensor.matmul(
                            ps[0:OC, :],
                            lhsT=wp_sb[:, k * OC:(k + 1) * OC],
                            rhs=rt[:, k * NQ:(k + 1) * NQ],
                            start=False, stop=(k == KT_E - 1),
                        )
                    yo = evac.tile([OC, NQ], f32, tag="yo")
                    nc.vector.tensor_scalar_add(yo[:], ps[0:OC, :],
                                                bp_sb[:, 0:1])
                    nc.sync.dma_start(yT[:, b * T + ns:b * T + ns + NQ],
                                      yo[:])

                quanta += [dma_r, mm_a, mm_b]
            return quanta

        # ---- attention with filler injection ------------------------------
        def attention(b, fillers, on_norm_done=None):
            prev = None

            def emit_fillers(k, reserve=4):
                # keep a few quanta in reserve so the pre-final-norm drain
                # has PE work to cover the DVE recip latency
                for _ in range(k):
                    if len(fillers) <= reserve:
                        return
                    fillers.pop(0)()

            def issue_evac_recip(state):
                # evacuate the PV pair to SBUF (frees its PSUM banks) and
                # start the reciprocal of the two denominator rows
                qt0, pv2, sb = state
                pvsb, rf, r16 = sb
                nc.vector.tensor_copy(pvsb[:], pv2[:])
                # approx-recip mishandles single-row APs at base partition
                # 64; run it over the whole 65-row slab (lanes independent,
                # junk in rows 0..63 never read)
                nc.vector.reciprocal_approx_fast(rf[0:hs + 1, :],
                                                 pvsb[0:hs + 1, :])
                nc.vector.tensor_copy(r16[hs:hs + 1, :], rf[hs:hs + 1, :])

            def issue_norm_rest(state):
                qt0, pv2, sb = state
                pvsb, rf, r16 = sb
                for hh in range(hpc):
                    bc = qkp.tile([P, NQ], f32, tag="qk", name="bc")
                    nc.tensor.matmul(
                        bc[0:hs, :],
                        lhsT=ones_sb[hs:hs + 1, :],
                        rhs=r16[hs:hs + 1, hh * NQ:(hh + 1) * NQ],
                        start=True,
                        stop=True,
                    )
                    ao = evac.tile([hs, NQ], fp16, tag="ao")
                    nc.vector.tensor_mul(
                        ao[:], bc[0:hs, :],
                        pvsb[0:hs, hh * NQ:(hh + 1) * NQ])
                    p, c = part_of(b, qt0 * NQ)
                    nc.sync.dma_start(
                        cc_ins[b][p][hh * hs:(hh + 1) * hs, c:c + NQ],
                        ao[:],
                    )
                if on_norm_done is not None:
                    p, c = part_of(b, qt0 * NQ)
                    if c + NQ == parts_w[b][p]:
                        on_norm_done(b, p)

            for qt in range(TQ):
                qs = b * T + qt * NQ
                nkt = DPB * qt + DPB
                pv2 = pvp.tile([hs + 1, NQ2], f32, tag="pv", name="pv2")
                pes = {}

                def issue_st(kt, qt=qt, qs=qs, pes=pes):
                    ks = b * T + kt * P
                    diag = kt >= DPB * qt
                    j = kt - DPB * qt
                    c0 = j * P if diag else 0
                    st2 = stp.tile([P, NQ2], f32, tag="st", name="st2")
                    for hh in range(hpc):
                        nc.tensor.matmul(
                            st2[:, hh * NQ + c0:(hh + 1) * NQ],
                            lhsT=KT[hh * hs:(hh + 1) * hs, ks:ks + P],
                            rhs=QT[hh * hs:(hh + 1) * hs,
                                   qs + c0:qs + NQ],
                            start=True,
                            stop=True,
                            tile_position=(hh * hs, 0),
                        )
                    pe2 = pexp.tile([P, NQ2], fp16, tag="pe", name="pe2")
                    if c0 == 0:
                        nc.scalar.activation(pe2[:], st2[:], Exp, scale=scale)
                    else:
                        nc.scalar.activation(
                            pe2[:].rearrange("p (h q) -> p h q",
                                             h=hpc)[:, :, c0:NQ],
                            st2[:].rearrange("p (h q) -> p h q",
                                             h=hpc)[:, :, c0:NQ],
                            Exp, scale=scale)
                    if diag:
                        # causal mask: zero the strictly-lower triangle of
                        # the diagonal [128,128] block on DVE (keeps the PE
                        # free of mask matmuls); masked exps contribute 0
                        # to both PV and the ones-column denominator
                        for hh in range(hpc):
                            blk = pe2[:, hh * NQ + c0:hh * NQ + c0 + P]
                            nc.vector.tensor_mul(blk, blk, mask_sb[:])
                    pes[kt] = (pe2, c0)

                def issue_pv(kt, nkt=nkt, pv2=pv2, pes=pes):
                    vi = b * TK + kt
                    pe2, c0 = pes.pop(kt)
                    for hh in range(hpc):
                        nc.tensor.matmul(
                            pv2[:, hh * NQ + c0:(hh + 1) * NQ],
                            lhsT=Vn[:, vi * WV + hh * (hs + 1):
                                    vi * WV + (hh + 1) * (hs + 1)],
                            rhs=pe2[:, hh * NQ + c0:(hh + 1) * NQ],
                            start=(kt == 0),
                            stop=(kt == nkt - 1),
                        )

                for kt in range(nkt):
                    issue_st(kt)
                    if kt == 3 and prev is not None:
                        issue_norm_rest(prev)
                    if kt >= 2:
                        issue_pv(kt - 2)
                    emit_fillers(1)
                for kt in range(max(0, nkt - 2), nkt):
                    issue_pv(kt)

                sb = (evac.tile([hs + 1, NQ2], f32, tag="pvs", name="pvsb",
                                bufs=2),
                      evac.tile([P, NQ2], f32, tag="rec", name="rf", bufs=2),
                      evac.tile([P, NQ2], fp16, tag="rec16", name="r16",
                                bufs=2))
                prev = (qt, pv2, sb)
                issue_evac_recip(prev)

            # give the DVE recip chain time to finish before the final
            # broadcast matmul enters the PE stream
            emit_fillers(4, reserve=0)
            issue_norm_rest(prev)

        # ---- main schedule ------------------------------------------------
        # prologue: batch 0's QKV dense, alternating PSUM slots
        for q in qkv_batch_quanta(0, [(stp, "st"), (qkp, "qk")]):
            q()

        for b in range(B):
            fillers = []
            if b + 1 < B:
                fillers += qkv_batch_quanta(b + 1, [(qkp, "qk")])
            if b >= 1:
                fillers += proj_quanta(b - 1, [(qkp, "qk")])
            attention(b, fillers, on_norm_done=issue_ag)
            # drain leftover fillers densely
            while fillers:
                fillers.pop(0)()
        for q in proj_quanta(B - 1, [(qkp, "qk")]):
            q()

    nc.compile()
    return nc


def shard_inputs(x, W_qkv, b_qkv, W_proj, b_proj, H, n_cores):
    B, T, C = x.shape
    hs = C // H
    hpc = H // n_cores
    CH = hpc * hs
    OC = C // n_cores
    x2 = np.asarray(x, dtype=np.float32).reshape(B * T, C)
    xT = np.ascontiguousarray(x2.T.astype(np.float16))
    W_qkv = np.asarray(W_qkv, dtype=np.float32)
    b_qkv = np.asarray(b_qkv, dtype=np.float32)
    W_proj = np.asarray(W_proj, dtype=np.float32)
    b_proj = np.asarray(b_proj, dtype=np.float32)
    in_maps = []
    for i in range(n_cores):
        sl = slice(i * CH, (i + 1) * CH)
        wqkv_i = np.ascontiguousarray(np.concatenate(
            [W_qkv[:, sl], W_qkv[:, C:][:, sl], W_qkv[:, 2 * C:][:, sl]],
            axis=1).astype(np.float16))
        bqkv_i = np.ascontiguousarray(np.stack(
            [b_qkv[sl], b_qkv[C:][sl], b_qkv[2 * C:][sl]], axis=1))
        wp_i = np.ascontiguousarray(
            W_proj[:, i * OC:(i + 1) * OC].astype(np.float16))
        bp_i = np.ascontiguousarray(b_proj[i * OC:(i + 1) * OC].reshape(OC, 1))
        in_maps.append({"xT": xT, "wqkv": wqkv_i, "bqkv": bqkv_i,
                        "wp": wp_i, "bp": bp_i})
    return in_maps


def gather_output(results, B, T, C, n_cores):
    yT = np.concatenate([results[i]["yT"] for i in range(n_cores)], axis=0)
    return np.ascontiguousarray(yT.T).reshape(B, T, C).astype(np.float32)


_NC_CACHE = {}


def _get_nc(B, T, C, H, n_cores):
    key = (B, T, C, H, n_cores)
    if key not in _NC_CACHE:
        _NC_CACHE[key] = build_attention_nc(B, T, C, H, n_cores)
    return _NC_CACHE[key]


def kernel(x, W_qkv, b_qkv, W_proj, b_proj):
    from concourse import bass_utils

    B, T, C = 4, 2048, 1024
    H, n_cores = 16, 8
    assert x.shape == (B, T, C)
    nc = _get_nc(B, T, C, H, n_cores)
    in_maps = shard_inputs(x, W_qkv, b_qkv, W_proj, b_proj, H, n_cores)
    res = bass_utils.run_bass_kernel_spmd(
        nc, in_maps, core_ids=list(range(n_cores))
    )
    return gather_output(res.results, B, T, C, n_cores)
